# revision 4
# baseline (speedup 1.0000x reference)
"""Trainium2 Bass kernel for a 2-layer Chebyshev KAN.

Computation (degree-5 Chebyshev KAN, matching the reference):
    t1  = tanh(x)
    y1  = sum_d T_d(tanh(t1)) @ C1_d.T + t1 @ Wb1.T + b1
    h   = SiLU(LayerNorm(y1))
    out = sum_d T_d(tanh(h)) @ C2_d.T + h @ Wb2.T + b2

Strategy: data-parallel over the batch dim across 8 NeuronCores (2048 rows =
16 partition tiles of 128 per core); weights replicated, pre-transposed +
cast to bf16 on the host and kept resident in SBUF.  The T_0 == 1 term is
folded into an effective bias on the host, leaving 6 [in,out] matmul
matrices per layer (base + d=1..5).

v2 pipeline: one 128-row tile at a time.  Per tile g the PE runs an
"A" segment (layer-1: 96 N=512 matmuls) and later a "B" segment (layer-2:
48 matmuls).  Segment order  A{0,1,2} A3 B0 A4 B1 ... A15 B12 B13 B14 B15;
the serial LayerNorm/SiLU chain of tile g and the Chebyshev feature
production (PE transpose in bf16 at 1 cyc/row + ACT tanh/square + DVE
recurrence) for upcoming segments are emitted as hooks inside the matmul
sweeps so every engine pipeline stays fed and the PE never idles.  The
first sweep covers three tiles so the weight-DMA arrival rate (~0.85us per
256KB block) stays ahead of PE consumption.  LayerNorm's scale/shift is
fused into the SiLU activation (per-partition scale/bias APs).
"""

import math

import numpy as np
import ml_dtypes

import concourse.bass as bass
import concourse.tile as tile
from concourse import bacc, mybir
from concourse.bass_utils import run_bass_kernel_spmd

N_CORES = 8
B, D0, D1, D2 = 16384, 1024, 1024, 512
BC = B // N_CORES            # rows per core
NT = BC // 128               # 16 partition tiles per core
LN_EPS = 1e-5

F32 = mybir.dt.float32
BF16 = mybir.dt.bfloat16
AF = mybir.ActivationFunctionType
ALU = mybir.AluOpType

SQRT2 = math.sqrt(2.0)


def _bcast_row(nc, pool, vec_ap, n, name, dtype=F32):
    """Load a [n] DRAM vector broadcast across all 128 partitions."""
    t = pool.tile([128, n], dtype, name=name)
    src = bass.AP(tensor=vec_ap.tensor, offset=vec_ap.offset,
                  ap=[[0, 128], list(vec_ap.ap[0])])
    nc.gpsimd.dma_start(out=t[:], in_=src)
    return t


def _rsqrt(nc, veps, statp, magic_t):
    """1/sqrt(veps) on DVE only (bit-trick seed + 2 Newton iterations).
    veps: [128, 1] f32 (> 0).  Avoids ACT Sqrt so the whole kernel stays on
    one activation table set."""
    I32 = mybir.dt.int32
    j = statp.tile([128, 1], I32, tag="rsj", name="rsj")
    nc.vector.tensor_scalar(j[:], veps[:].bitcast(I32), 1, None,
                            op0=ALU.arith_shift_right)
    y = statp.tile([128, 1], F32, tag="rsy", name="rsy")
    nc.vector.tensor_tensor(y[:].bitcast(I32), magic_t[:], j[:], op=ALU.subtract)
    s = statp.tile([128, 1], F32, tag="rss", name="rss")
    w = statp.tile([128, 1], F32, tag="rsw", name="rsw")
    for _ in range(2):
        nc.vector.tensor_tensor(s[:], y[:], y[:], op=ALU.mult)
        nc.vector.tensor_tensor(s[:], s[:], veps[:], op=ALU.mult)
        nc.vector.tensor_scalar(w[:], s[:], -0.5, 1.5, op0=ALU.mult, op1=ALU.add)
        nc.vector.tensor_tensor(y[:], y[:], w[:], op=ALU.mult)
    return y


def _kernel_body(tc, out_d, x_d, w1_d, w2_d, b1_d, b2_d, g_d, be_d):
    nc = tc.nc
    trivial = g_d is None
    import contextlib
    ctx = contextlib.ExitStack()
    with ctx:
        consts = ctx.enter_context(tc.tile_pool(name="consts", bufs=1))
        wpool = ctx.enter_context(tc.tile_pool(name="wpool", bufs=1))
        xpool = ctx.enter_context(tc.tile_pool(name="xpool", bufs=3))
        xbp = ctx.enter_context(tc.tile_pool(name="xbp", bufs=4))
        hbp = ctx.enter_context(tc.tile_pool(name="hbp", bufs=3))
        y1p = ctx.enter_context(tc.tile_pool(name="y1p", bufs=2))
        chebp = ctx.enter_context(tc.tile_pool(name="chebp", bufs=8))
        upool = ctx.enter_context(tc.tile_pool(name="upool", bufs=8))
        statp = ctx.enter_context(tc.tile_pool(name="statp", bufs=8))
        opool = ctx.enter_context(tc.tile_pool(name="opool", bufs=2))
        ps_y1 = ctx.enter_context(tc.tile_pool(name="ps_y1", bufs=4, space="PSUM"))
        ps_b = ctx.enter_context(tc.tile_pool(name="ps_b", bufs=2, space="PSUM"))
        ps_tr = ctx.enter_context(tc.tile_pool(name="ps_tr", bufs=2, space="PSUM"))

        ident = consts.tile([128, 128], BF16, name="ident")
        ident_dram = nc.inline_tensor(
            np.eye(128, dtype=np.float32).astype(ml_dtypes.bfloat16),
            name="ident_dram")
        nc.sync.dma_start(out=ident[:], in_=ident_dram.ap())
        magic_t = consts.tile([128, 1], mybir.dt.int32, name="magic_t")
        nc.vector.memset(magic_t[:], 0x5F3759DF)
        # Trigger the (single) ACT table-set load while the first DMAs are in
        # flight: Silu selects silu_and_others, which also covers Tanh/Square/
        # Copy -- the only ACT functions this kernel uses.
        warm = consts.tile([128, 1], F32, name="warm")
        nc.scalar.activation(warm[:], magic_t[:].bitcast(F32), AF.Silu)

        b1_t = _bcast_row(nc, consts, b1_d, D1, "b1_t")
        b2_t = _bcast_row(nc, consts, b2_d, D2, "b2_t")
        g_t = (None if trivial else _bcast_row(nc, consts, g_d, D1, "g_t", BF16))
        be_t = (None if trivial else _bcast_row(nc, consts, be_d, D1, "be_t", BF16))

        # ------------------------------------------------------------------
        # bookkeeping
        x_tiles = {}      # g -> (f32 tile, n_slices)
        xb_tiles = {}     # g -> bf16 tanh(x) tile
        hb_tiles = {}     # g -> bf16 silu(ln(y1)) tile
        y1ps = {}         # g -> [h0 bank, h1 bank]
        y2ps = {}         # g -> bank
        cheb_pre = {}     # (tag, g, i) -> cheb tile  (prefilled)

        def dma_x(g, slices=1):
            x_t = xpool.tile([128, D0], F32, tag="x", name=f"x_{g}")
            w = D0 // slices
            for q in range(slices):
                nc.sync.dma_start(
                    out=x_t[:, q * w:(q + 1) * w],
                    in_=x_d[g * 128:(g + 1) * 128, q * w:(q + 1) * w])
            x_tiles[g] = x_t

        def tanh_x(g, slices=1):
            x_t = x_tiles.pop(g)
            xb = xbp.tile([128, D0], BF16, tag="xb", name=f"xb_{g}")
            w = D0 // slices
            for q in range(slices):
                nc.scalar.activation(xb[:, q * w:(q + 1) * w],
                                     x_t[:, q * w:(q + 1) * w], AF.Tanh)
            xb_tiles[g] = xb

        def fill(src, tag, g, i):
            """Produce the 6 stationary feature blocks for (tile g, i-block i)
            from batch-major bf16 activations `src`:
              cheb[:,0] = src_block.T (base feature), cheb[:,1] = tanh(.) = T1,
              cheb[:,2..5] = T2..T5 via the Chebyshev recurrence.
            PE bf16 transpose (1 cyc/row) + 2 ACT Squares + 7 DVE ops."""
            cheb = chebp.tile([128, 6, 128], BF16, tag="cheb",
                              name=f"cb{tag}{g}_{i}")
            tr = ps_tr.tile([128, 128], BF16, tag="tr", name=f"tr{tag}{g}_{i}")
            nc.tensor.transpose(tr[:], src[:, i * 128:(i + 1) * 128], ident[:])
            nc.scalar.copy(cheb[:, 0], tr[:])
            nc.scalar.activation(cheb[:, 1], tr[:], AF.Tanh)
            T1, T2, T3, T4, T5 = (cheb[:, k] for k in range(1, 6))
            sq = upool.tile([128, 128], BF16, tag="u", name="sq")
            nc.scalar.activation(sq[:], T1, AF.Square, scale=SQRT2)  # 2*T1^2
            nc.vector.tensor_scalar(T2, sq[:], 1.0, None, op0=ALU.subtract)
            a = upool.tile([128, 128], BF16, tag="u", name="a")
            nc.vector.tensor_scalar(a[:], T2, 2.0, 1.0, op0=ALU.mult,
                                    op1=ALU.subtract)                # 2*T2-1
            nc.vector.tensor_tensor(T3, T1, a[:], op=ALU.mult)
            sq2 = upool.tile([128, 128], BF16, tag="u", name="sq2")
            nc.scalar.activation(sq2[:], T2, AF.Square, scale=SQRT2)  # 2*T2^2
            nc.vector.tensor_scalar(T4, sq2[:], 1.0, None, op0=ALU.subtract)
            c = upool.tile([128, 128], BF16, tag="u", name="c")
            nc.vector.tensor_tensor(c[:], T2, T3, op=ALU.mult)
            d = upool.tile([128, 128], BF16, tag="u", name="d")
            nc.vector.tensor_scalar(d[:], c[:], 2.0, None, op0=ALU.mult)
            nc.vector.tensor_tensor(T5, d[:], T1, op=ALU.subtract)
            return cheb

        def emit_LN(g):
            """bias + LayerNorm stats + (fused) SiLU for tile g -> hb bf16."""
            ps = y1ps.pop(g)
            y1 = y1p.tile([128, D1], F32, tag="y1", name=f"y1_{g}")
            for h in range(2):
                sl = slice(h * 512, (h + 1) * 512)
                nc.vector.tensor_add(y1[:, sl], ps[h][:], b1_t[:, sl])
            stats = statp.tile([128, 2, 6], F32, tag="stats", name="stats")
            nc.vector.bn_stats(stats[:, 0, :], y1[:, 0:512])
            nc.vector.bn_stats(stats[:, 1, :], y1[:, 512:1024])
            mv = statp.tile([128, 2], F32, tag="mv", name="mv")
            nc.vector.bn_aggr(mv[:], stats[:])
            veps = statp.tile([128, 1], F32, tag="veps", name="veps")
            nc.vector.tensor_scalar(veps[:], mv[:, 1:2], LN_EPS, None,
                                    op0=ALU.add)
            rstd = _rsqrt(nc, veps, statp, magic_t)
            hb = hbp.tile([128, D1], BF16, tag="hb", name=f"hb_{g}")
            if trivial:
                # silu((y1 - mu) * rstd) == Silu(y1*rstd + (-mu*rstd)) fused
                # into one ACT op with per-partition scale/bias vectors.
                nmr = statp.tile([128, 1], F32, tag="nmr", name="nmr")
                nc.vector.tensor_scalar(nmr[:], mv[:, 0:1], -1.0, None,
                                        op0=ALU.mult)
                nc.vector.tensor_tensor(nmr[:], nmr[:], rstd[:], op=ALU.mult)
                nc.scalar.activation(hb[:], y1[:], AF.Silu, bias=nmr[:],
                                     scale=rstd[:])
            else:
                nc.vector.tensor_scalar(y1[:], y1[:], mv[:, 0:1], rstd[:],
                                        op0=ALU.subtract, op1=ALU.mult)
                nc.vector.tensor_mul(y1[:], y1[:], g_t[:])
                nc.vector.tensor_add(y1[:], y1[:], be_t[:])
                nc.scalar.activation(hb[:], y1[:], AF.Silu)
            hb_tiles[g] = hb

        def emit_evac(k):
            ps = y2ps.pop(k)
            o = opool.tile([128, D2], F32, tag="o", name=f"o_{k}")
            nc.vector.tensor_add(o[:], ps[:], b2_t[:])
            nc.sync.dma_start(out=out_d[k * 128:(k + 1) * 128, :], in_=o[:])

        def run_hooks(hooks, i):
            for fn in hooks.get(i, ()):
                fn()

        def emit_A(g_list, hooks, fill_ahead=2):
            """Layer-1 sweep over tiles in g_list (usually one tile)."""
            for gi, g in enumerate(g_list):
                pool = ps_b if (len(g_list) > 1 and gi == 2) else ps_y1
                tg = "b" if (len(g_list) > 1 and gi == 2) else "y1"
                y1ps[g] = [pool.tile([128, 512], F32, tag=tg,
                                     name=f"y1ps_{g}_{h}") for h in range(2)]
            chebs = {g: {} for g in g_list}
            for g in g_list:
                for i in range(min(fill_ahead, 8)):
                    key = ("A", g, i)
                    chebs[g][i] = (cheb_pre.pop(key) if key in cheb_pre
                                   else fill(xb_tiles[g], "A", g, i))
            for i in range(8):
                run_hooks(hooks, i)
                for g in g_list:
                    if i + fill_ahead < 8:
                        chebs[g][i + fill_ahead] = fill(xb_tiles[g], "A", g,
                                                        i + fill_ahead)
                for d in range(6):
                    for g in g_list:
                        st = chebs[g][i][:, d, :]
                        for h in range(2):
                            nc.tensor.matmul(
                                y1ps[g][h][:], st,
                                w1_sb[:, d, i, h * 512:(h + 1) * 512],
                                start=(i == 0 and d == 0),
                                stop=(i == 7 and d == 5))
            for g in g_list:
                xb_tiles.pop(g, None)
            run_hooks(hooks, 8)

        def emit_B(k, hooks):
            """Layer-2 sweep for tile k (input hb_tiles[k])."""
            y2 = ps_b.tile([128, 512], F32, tag="b", name=f"y2ps_{k}")
            y2ps[k] = y2
            chebs = {}
            for i in range(2):
                key = ("B", k, i)
                chebs[i] = (cheb_pre.pop(key) if key in cheb_pre
                            else fill(hb_tiles[k], "B", k, i))
            for i in range(8):
                run_hooks(hooks, i)
                if i + 2 < 8:
                    chebs[i + 2] = fill(hb_tiles[k], "B", k, i + 2)
                for d in range(6):
                    nc.tensor.matmul(y2[:], chebs[i][:, d, :],
                                     w2_sb[:, d, i, :],
                                     start=(i == 0 and d == 0),
                                     stop=(i == 7 and d == 5))
            hb_tiles.pop(k, None)
            run_hooks(hooks, 8)

        # ------------------------------------------------------------------
        # startup DMAs: first x tiles sliced fine so the ACT/transpose chain
        # starts within ~2us; weights i-block-ordered to match consumption.
        dma_x(0, slices=4)
        dma_x(1, slices=2)

        w1_sb = wpool.tile([128, 6, 8, D1], BF16, name="w1_sb")
        w2_sb = wpool.tile([128, 6, 8, D2], BF16, name="w2_sb")
        for d in range(6):
            src = w1_d[d, 0:128, :]
            for q in range(2):
                nc.sync.dma_start(out=w1_sb[:, d, 0, q * 512:(q + 1) * 512],
                                  in_=src[:, q * 512:(q + 1) * 512])
        dma_x(2)
        for i in range(1, 8):
            for d in range(6):
                nc.sync.dma_start(out=w1_sb[:, d, i, :],
                                  in_=w1_d[d, i * 128:(i + 1) * 128, :])
        for i in range(8):
            for d in range(6):
                nc.sync.dma_start(out=w2_sb[:, d, i, :],
                                  in_=w2_d[d, i * 128:(i + 1) * 128, :])

        tanh_x(0, slices=4)
        tanh_x(1, slices=2)
        tanh_x(2)
        for g in range(3):
            cheb_pre[("A", g, 0)] = fill(xb_tiles[g], "A", g, 0)

        # ------------------------------------------------------------------
        # segment schedule: S0=A{0,1,2}, A3, B0, A4, B1, ..., A15, B12..B15
        emit_A([0, 1, 2], fill_ahead=1, hooks={
            0: [lambda: dma_x(3)],
            4: [lambda: tanh_x(3), lambda: dma_x(4)],
            6: [lambda: cheb_pre.__setitem__(("A", 3, 0),
                                             fill(xb_tiles[3], "A", 3, 0))],
            7: [lambda: cheb_pre.__setitem__(("A", 3, 1),
                                             fill(xb_tiles[3], "A", 3, 1))],
            8: [lambda: emit_LN(0)],
        })

        def A_hooks(g):
            h = {}
            add = lambda i, fn: h.setdefault(i, []).append(fn)
            if g == 3:
                add(1, lambda: emit_LN(1))
                add(4, lambda: emit_LN(2))
            else:
                add(1, lambda: emit_LN(g - 1))
            if g >= 4:
                add(2, lambda: emit_evac(g - 4))
            if g + 1 <= 15:
                add(4, lambda: tanh_x(g + 1))
            if g + 2 <= 15:
                add(4, lambda: dma_x(g + 2))
            # prefill for B_{g-3}, which directly follows this segment
            add(6, lambda: cheb_pre.__setitem__(
                ("B", g - 3, 0), fill(hb_tiles[g - 3], "B", g - 3, 0)))
            add(7, lambda: cheb_pre.__setitem__(
                ("B", g - 3, 1), fill(hb_tiles[g - 3], "B", g - 3, 1)))
            return h

        def B_hooks(k):
            h = {}
            add = lambda i, fn: h.setdefault(i, []).append(fn)
            if k + 4 <= 15:
                # prefill for A_{k+4}, which directly follows this segment
                add(6, lambda: cheb_pre.__setitem__(
                    ("A", k + 4, 0), fill(xb_tiles[k + 4], "A", k + 4, 0)))
                add(7, lambda: cheb_pre.__setitem__(
                    ("A", k + 4, 1), fill(xb_tiles[k + 4], "A", k + 4, 1)))
            if k == 12:
                add(4, lambda: cheb_pre.__setitem__(
                    ("B", 13, 0), fill(hb_tiles[13], "B", 13, 0)))
                add(5, lambda: cheb_pre.__setitem__(
                    ("B", 13, 1), fill(hb_tiles[13], "B", 13, 1)))
            if k == 13:
                # LN(15) here (not at B12): writing hb(15) recycles hb(12)'s
                # slot, whose readers (B12's fills) must all be emitted first.
                add(1, lambda: emit_LN(15))
            if k in (13, 14):
                add(6, lambda: cheb_pre.__setitem__(
                    ("B", k + 1, 0), fill(hb_tiles[k + 1], "B", k + 1, 0)))
                add(7, lambda: cheb_pre.__setitem__(
                    ("B", k + 1, 1), fill(hb_tiles[k + 1], "B", k + 1, 1)))
            if k >= 13:
                add(2, lambda: emit_evac(k - 1))
            if k == 15:
                add(8, lambda: emit_evac(15))
            return h

        emit_A([3], A_hooks(3))
        for k in range(12):
            emit_B(k, B_hooks(k))
            emit_A([k + 4], A_hooks(k + 4))
        for k in range(12, 16):
            emit_B(k, B_hooks(k))


_PROGRAMS = {}


def _get_program(trivial_affine: bool):
    key = trivial_affine
    if key in _PROGRAMS:
        return _PROGRAMS[key]
    nc = bacc.Bacc("TRN2", target_bir_lowering=False, debug=False,
                   num_devices=N_CORES)
    x_d = nc.dram_tensor("x_in", [BC, D0], F32, kind="ExternalInput").ap()
    w1_d = nc.dram_tensor("w1", [6, D0, D1], BF16, kind="ExternalInput").ap()
    w2_d = nc.dram_tensor("w2", [6, D1, D2], BF16, kind="ExternalInput").ap()
    b1_d = nc.dram_tensor("b1e", [D1], F32, kind="ExternalInput").ap()
    b2_d = nc.dram_tensor("b2e", [D2], F32, kind="ExternalInput").ap()
    if trivial_affine:
        g_d = be_d = None
    else:
        g_d = nc.dram_tensor("gam", [D1], BF16, kind="ExternalInput").ap()
        be_d = nc.dram_tensor("bet", [D1], BF16, kind="ExternalInput").ap()
    out_d = nc.dram_tensor("out", [BC, D2], F32, kind="ExternalOutput").ap()

    with tile.TileContext(nc) as tc:
        _kernel_body(tc, out_d, x_d, w1_d, w2_d, b1_d, b2_d, g_d, be_d)
    nc.compile()
    _PROGRAMS[key] = nc
    return nc


def _prep_inputs(x, coeff1, base_w1, bias1, ln_gamma, ln_beta, coeff2,
                 base_w2, bias2):
    x = np.ascontiguousarray(np.asarray(x, np.float32))
    coeff1 = np.asarray(coeff1, np.float32)
    coeff2 = np.asarray(coeff2, np.float32)

    w1 = np.empty((6, D0, D1), ml_dtypes.bfloat16)
    w1[0] = np.asarray(base_w1, np.float32).T
    for d in range(1, 6):
        w1[d] = coeff1[:, :, d].T
    w2 = np.empty((6, D1, D2), ml_dtypes.bfloat16)
    w2[0] = np.asarray(base_w2, np.float32).T
    for d in range(1, 6):
        w2[d] = coeff2[:, :, d].T
    b1e = (np.asarray(bias1, np.float32)
           + coeff1[:, :, 0].sum(axis=1)).astype(np.float32)
    b2e = (np.asarray(bias2, np.float32)
           + coeff2[:, :, 0].sum(axis=1)).astype(np.float32)

    g = np.asarray(ln_gamma, np.float32)
    be = np.asarray(ln_beta, np.float32)
    trivial = bool(np.all(g == 1.0) and np.all(be == 0.0))

    shared = {"w1": w1, "w2": w2, "b1e": b1e, "b2e": b2e}
    if not trivial:
        shared["gam"] = g.astype(ml_dtypes.bfloat16)
        shared["bet"] = be.astype(ml_dtypes.bfloat16)
    in_maps = []
    for cid in range(N_CORES):
        m = dict(shared)
        m["x_in"] = np.ascontiguousarray(x[cid * BC:(cid + 1) * BC])
        in_maps.append(m)
    return trivial, in_maps


def kernel_run(trace=False, **inputs):
    trivial, in_maps = _prep_inputs(**inputs)
    nc = _get_program(trivial)
    res = run_bass_kernel_spmd(nc, in_maps, core_ids=list(range(N_CORES)),
                               trace=trace)
    out = np.concatenate([r["out"] for r in res.results], axis=0)
    return out, res


def kernel(**inputs):
    out, _ = kernel_run(trace=False, **inputs)
    return out


# revision 13
# speedup vs baseline: 1.0690x; 1.0690x over previous
"""Trainium2 Bass kernel for a 2-layer Chebyshev KAN.

Computation (degree-5 Chebyshev KAN, matching the reference):
    t1  = tanh(x)
    y1  = sum_d T_d(tanh(t1)) @ C1_d.T + t1 @ Wb1.T + b1
    h   = SiLU(LayerNorm(y1))
    out = sum_d T_d(tanh(h)) @ C2_d.T + h @ Wb2.T + b2

Strategy: data-parallel over the batch dim across 8 NeuronCores (2048 rows =
16 partition tiles of 128 per core); weights replicated, pre-transposed +
cast to bf16 on the host and kept resident in SBUF.  The T_0 == 1 term is
folded into an effective bias on the host, leaving 6 [in,out] matmul
matrices per layer (base + d=1..5).

v2 pipeline: one 128-row tile at a time.  Per tile g the PE runs an
"A" segment (layer-1: 96 N=512 matmuls) and later a "B" segment (layer-2:
48 matmuls).  Segment order  A{0,1,2} A3 B0 A4 B1 ... A15 B12 B13 B14 B15;
the serial LayerNorm/SiLU chain of tile g and the Chebyshev feature
production (PE transpose in bf16 at 1 cyc/row + ACT tanh/square + DVE
recurrence) for upcoming segments are emitted as hooks inside the matmul
sweeps so every engine pipeline stays fed and the PE never idles.  The
first sweep covers three tiles so the weight-DMA arrival rate (~0.85us per
256KB block) stays ahead of PE consumption.  LayerNorm's scale/shift is
fused into the SiLU activation (per-partition scale/bias APs).
"""

import math

import numpy as np
import ml_dtypes

import concourse.bass as bass
import concourse.tile as tile
from concourse import bacc, mybir
from concourse.bass_utils import run_bass_kernel_spmd

N_CORES = 8
B, D0, D1, D2 = 16384, 1024, 1024, 512
BC = B // N_CORES            # rows per core
NT = BC // 128               # 16 partition tiles per core
LN_EPS = 1e-5

F32 = mybir.dt.float32
BF16 = mybir.dt.bfloat16
AF = mybir.ActivationFunctionType
ALU = mybir.AluOpType

SQRT2 = math.sqrt(2.0)


def _bcast_row(nc, pool, vec_ap, n, name, dtype=F32):
    """Load a [n] DRAM vector broadcast across all 128 partitions."""
    t = pool.tile([128, n], dtype, name=name)
    src = bass.AP(tensor=vec_ap.tensor, offset=vec_ap.offset,
                  ap=[[0, 128], list(vec_ap.ap[0])])
    nc.gpsimd.dma_start(out=t[:], in_=src)
    return t


def _rsqrt(nc, veps, statp, magic_t):
    """1/sqrt(veps) on DVE only (bit-trick seed + 2 Newton iterations).
    veps: [128, 1] f32 (> 0).  Avoids ACT Sqrt so the whole kernel stays on
    one activation table set."""
    I32 = mybir.dt.int32
    j = statp.tile([128, 1], I32, tag="rsj", name="rsj")
    nc.vector.tensor_scalar(j[:], veps[:].bitcast(I32), 1, None,
                            op0=ALU.arith_shift_right)
    y = statp.tile([128, 1], F32, tag="rsy", name="rsy")
    nc.vector.tensor_tensor(y[:].bitcast(I32), magic_t[:], j[:], op=ALU.subtract)
    s = statp.tile([128, 1], F32, tag="rss", name="rss")
    w = statp.tile([128, 1], F32, tag="rsw", name="rsw")
    for _ in range(2):
        nc.vector.tensor_tensor(s[:], y[:], y[:], op=ALU.mult)
        nc.vector.tensor_tensor(s[:], s[:], veps[:], op=ALU.mult)
        nc.vector.tensor_scalar(w[:], s[:], -0.5, 1.5, op0=ALU.mult, op1=ALU.add)
        nc.vector.tensor_tensor(y[:], y[:], w[:], op=ALU.mult)
    return y


def _kernel_body(tc, out_d, x_d, w1_d, w2_d, b1_d, b2_d, g_d, be_d):
    nc = tc.nc
    trivial = g_d is None
    import contextlib
    ctx = contextlib.ExitStack()
    with ctx:
        consts = ctx.enter_context(tc.tile_pool(name="consts", bufs=1))
        wpool = ctx.enter_context(tc.tile_pool(name="wpool", bufs=1))
        xpool = ctx.enter_context(tc.tile_pool(name="xpool", bufs=3))
        xbp = ctx.enter_context(tc.tile_pool(name="xbp", bufs=4))
        hbp = ctx.enter_context(tc.tile_pool(name="hbp", bufs=3))
        y1p = ctx.enter_context(tc.tile_pool(name="y1p", bufs=2))
        chebp = ctx.enter_context(tc.tile_pool(name="chebp", bufs=8))
        upool = ctx.enter_context(tc.tile_pool(name="upool", bufs=8))
        statp = ctx.enter_context(tc.tile_pool(name="statp", bufs=8))
        opool = ctx.enter_context(tc.tile_pool(name="opool", bufs=2))
        ps_y1 = ctx.enter_context(tc.tile_pool(name="ps_y1", bufs=4, space="PSUM"))
        ps_b = ctx.enter_context(tc.tile_pool(name="ps_b", bufs=2, space="PSUM"))
        ps_tr = ctx.enter_context(tc.tile_pool(name="ps_tr", bufs=2, space="PSUM"))

        ident = consts.tile([128, 128], BF16, name="ident")
        ident_dram = nc.inline_tensor(
            np.eye(128, dtype=np.float32).astype(ml_dtypes.bfloat16),
            name="ident_dram")
        nc.sync.dma_start(out=ident[:], in_=ident_dram.ap())
        magic_t = consts.tile([128, 1], mybir.dt.int32, name="magic_t")
        nc.vector.memset(magic_t[:], 0x5F3759DF)
        # Trigger the (single) ACT table-set load while the first DMAs are in
        # flight: Silu selects silu_and_others, which also covers Tanh/Square/
        # Copy -- the only ACT functions this kernel uses.
        warm = consts.tile([128, 1], F32, name="warm")
        nc.scalar.activation(warm[:], magic_t[:].bitcast(F32), AF.Silu)

        b1_t = _bcast_row(nc, consts, b1_d, D1, "b1_t")
        b2_t = _bcast_row(nc, consts, b2_d, D2, "b2_t")
        g_t = (None if trivial else _bcast_row(nc, consts, g_d, D1, "g_t", BF16))
        be_t = (None if trivial else _bcast_row(nc, consts, be_d, D1, "be_t", BF16))

        # ------------------------------------------------------------------
        # bookkeeping
        x_tiles = {}      # g -> (f32 tile, n_slices)
        xb_tiles = {}     # g -> bf16 tanh(x) tile
        hb_tiles = {}     # g -> bf16 silu(ln(y1)) tile
        y1ps = {}         # g -> [h0 bank, h1 bank]
        y2ps = {}         # g -> bank
        cheb_pre = {}     # (tag, g, i) -> cheb tile  (prefilled)

        def dma_x(g, slices=1):
            # gpsimd issue queue: keeps x tiles off the (weight-heavy) sync
            # queue so they cannot be delayed behind 19MB of weight traffic.
            x_t = xpool.tile([128, D0], F32, tag="x", name=f"x_{g}")
            w = D0 // slices
            for q in range(slices):
                nc.gpsimd.dma_start(
                    out=x_t[:, q * w:(q + 1) * w],
                    in_=x_d[g * 128:(g + 1) * 128, q * w:(q + 1) * w])
            x_tiles[g] = x_t

        def tanh_x(g, slices=1):
            x_t = x_tiles.pop(g)
            xb = xbp.tile([128, D0], BF16, tag="xb", name=f"xb_{g}")
            w = D0 // slices
            for q in range(slices):
                nc.scalar.activation(xb[:, q * w:(q + 1) * w],
                                     x_t[:, q * w:(q + 1) * w], AF.Tanh)
            xb_tiles[g] = xb

        def fill(src, tag, g, i):
            """Produce the 6 stationary feature blocks for (tile g, i-block i)
            from batch-major bf16 activations `src`:
              cheb[:,0] = src_block.T (base feature), cheb[:,1] = tanh(.) = T1,
              cheb[:,2..5] = T2..T5 via the Chebyshev recurrence.
            PE bf16 transpose (1 cyc/row) + 2 ACT Squares + 7 DVE ops."""
            cheb = chebp.tile([128, 6, 128], BF16, tag="cheb",
                              name=f"cb{tag}{g}_{i}")
            tr = ps_tr.tile([128, 128], BF16, tag="tr", name=f"tr{tag}{g}_{i}")
            nc.tensor.transpose(tr[:], src[:, i * 128:(i + 1) * 128], ident[:])
            nc.scalar.copy(cheb[:, 0], tr[:])
            nc.scalar.activation(cheb[:, 1], tr[:], AF.Tanh)
            T1, T2, T3, T4, T5 = (cheb[:, k] for k in range(1, 6))
            sq = upool.tile([128, 128], BF16, tag="u", name="sq")
            nc.scalar.activation(sq[:], T1, AF.Square, scale=SQRT2)  # 2*T1^2
            nc.vector.tensor_scalar(T2, sq[:], 1.0, None, op0=ALU.subtract)
            a = upool.tile([128, 128], BF16, tag="u", name="a")
            nc.vector.tensor_scalar(a[:], T2, 2.0, 1.0, op0=ALU.mult,
                                    op1=ALU.subtract)                # 2*T2-1
            nc.vector.tensor_tensor(T3, T1, a[:], op=ALU.mult)
            sq2 = upool.tile([128, 128], BF16, tag="u", name="sq2")
            nc.scalar.activation(sq2[:], T2, AF.Square, scale=SQRT2)  # 2*T2^2
            nc.vector.tensor_scalar(T4, sq2[:], 1.0, None, op0=ALU.subtract)
            c = upool.tile([128, 128], BF16, tag="u", name="c")
            nc.vector.tensor_tensor(c[:], T2, T3, op=ALU.mult)
            d = upool.tile([128, 128], BF16, tag="u", name="d")
            nc.vector.tensor_scalar(d[:], c[:], 2.0, None, op0=ALU.mult)
            nc.vector.tensor_tensor(T5, d[:], T1, op=ALU.subtract)
            return cheb

        def emit_LN(g):
            """bias + LayerNorm stats + (fused) SiLU for tile g -> hb bf16."""
            ps = y1ps.pop(g)
            y1 = y1p.tile([128, D1], F32, tag="y1", name=f"y1_{g}")
            for h in range(2):
                sl = slice(h * 512, (h + 1) * 512)
                nc.vector.tensor_add(y1[:, sl], ps[h][:], b1_t[:, sl])
            stats = statp.tile([128, 2, 6], F32, tag="stats", name="stats")
            nc.vector.bn_stats(stats[:, 0, :], y1[:, 0:512])
            nc.vector.bn_stats(stats[:, 1, :], y1[:, 512:1024])
            mv = statp.tile([128, 2], F32, tag="mv", name="mv")
            nc.vector.bn_aggr(mv[:], stats[:])
            veps = statp.tile([128, 1], F32, tag="veps", name="veps")
            nc.vector.tensor_scalar(veps[:], mv[:, 1:2], LN_EPS, None,
                                    op0=ALU.add)
            rstd = _rsqrt(nc, veps, statp, magic_t)
            hb = hbp.tile([128, D1], BF16, tag="hb", name=f"hb_{g}")
            if trivial:
                # silu((y1 - mu) * rstd) == Silu(y1*rstd + (-mu*rstd)) fused
                # into one ACT op with per-partition scale/bias vectors.
                nmr = statp.tile([128, 1], F32, tag="nmr", name="nmr")
                nc.vector.tensor_scalar(nmr[:], mv[:, 0:1], -1.0, None,
                                        op0=ALU.mult)
                nc.vector.tensor_tensor(nmr[:], nmr[:], rstd[:], op=ALU.mult)
                nc.scalar.activation(hb[:], y1[:], AF.Silu, bias=nmr[:],
                                     scale=rstd[:])
            else:
                nc.vector.tensor_scalar(y1[:], y1[:], mv[:, 0:1], rstd[:],
                                        op0=ALU.subtract, op1=ALU.mult)
                nc.vector.tensor_mul(y1[:], y1[:], g_t[:])
                nc.vector.tensor_add(y1[:], y1[:], be_t[:])
                nc.scalar.activation(hb[:], y1[:], AF.Silu)
            hb_tiles[g] = hb

        def emit_evac(k):
            ps = y2ps.pop(k)
            o = opool.tile([128, D2], F32, tag="o", name=f"o_{k}")
            nc.vector.tensor_add(o[:], ps[:], b2_t[:])
            nc.sync.dma_start(out=out_d[k * 128:(k + 1) * 128, :], in_=o[:])

        def run_hooks(hooks, i):
            for fn in hooks.get(i, ()):
                fn()

        def emit_A(g_list, hooks, fill_ahead=2):
            """Layer-1 sweep over tiles in g_list (usually one tile)."""
            for gi, g in enumerate(g_list):
                pool = ps_b if (len(g_list) > 1 and gi == 2) else ps_y1
                tg = "b" if (len(g_list) > 1 and gi == 2) else "y1"
                y1ps[g] = [pool.tile([128, 512], F32, tag=tg,
                                     name=f"y1ps_{g}_{h}") for h in range(2)]
            chebs = {g: {} for g in g_list}
            for g in g_list:
                for i in range(min(fill_ahead, 8)):
                    key = ("A", g, i)
                    chebs[g][i] = (cheb_pre.pop(key) if key in cheb_pre
                                   else fill(xb_tiles[g], "A", g, i))
            for i in range(8):
                run_hooks(hooks, i)
                for g in g_list:
                    if i + fill_ahead < 8:
                        chebs[g][i + fill_ahead] = fill(xb_tiles[g], "A", g,
                                                        i + fill_ahead)
                for d in range(6):
                    for g in g_list:
                        st = chebs[g][i][:, d, :]
                        for h in range(2):
                            nc.tensor.matmul(
                                y1ps[g][h][:], st,
                                w1_sb[:, i, d, h * 512:(h + 1) * 512],
                                start=(i == 0 and d == 0),
                                stop=(i == 7 and d == 5))
            for g in g_list:
                xb_tiles.pop(g, None)
            run_hooks(hooks, 8)

        def emit_B(k, hooks):
            """Layer-2 sweep for tile k (input hb_tiles[k])."""
            y2 = ps_b.tile([128, 512], F32, tag="b", name=f"y2ps_{k}")
            y2ps[k] = y2
            chebs = {}
            for i in range(2):
                key = ("B", k, i)
                chebs[i] = (cheb_pre.pop(key) if key in cheb_pre
                            else fill(hb_tiles[k], "B", k, i))
            for i in range(8):
                run_hooks(hooks, i)
                if i + 2 < 8:
                    chebs[i + 2] = fill(hb_tiles[k], "B", k, i + 2)
                for d in range(6):
                    nc.tensor.matmul(y2[:], chebs[i][:, d, :],
                                     w2_sb[:, i, d, :],
                                     start=(i == 0 and d == 0),
                                     stop=(i == 7 and d == 5))
            hb_tiles.pop(k, None)
            run_hooks(hooks, 8)

        # ------------------------------------------------------------------
        # startup DMAs: first x tiles sliced fine so the ACT/transpose chain
        # starts within ~2us; weights i-block-ordered to match consumption.
        dma_x(0, slices=4)
        dma_x(1, slices=2)

        # Weights arrive i-block-major (host pre-permuted): one large DMA per
        # 1.5MB i-block (12KB per partition line) so the weight load is not
        # issue-rate limited; w1 i=0 is split per-d (d=0 halved) so the very
        # first matmuls aren't gated on a full block.
        w1_sb = wpool.tile([128, 8, 6, D1], BF16, name="w1_sb")
        w2_sb = wpool.tile([128, 8, 6, D2], BF16, name="w2_sb")
        for d in range(6):
            src = w1_d[0, :, d, :]
            if d == 0:
                for q in range(2):
                    nc.sync.dma_start(out=w1_sb[:, 0, 0, q * 512:(q + 1) * 512],
                                      in_=src[:, q * 512:(q + 1) * 512])
            else:
                nc.sync.dma_start(out=w1_sb[:, 0, d, :], in_=src)
        dma_x(2)
        for i in range(1, 8):
            nc.sync.dma_start(out=w1_sb[:, i], in_=w1_d[i])
        for i in range(8):
            nc.sync.dma_start(out=w2_sb[:, i], in_=w2_d[i])

        tanh_x(0, slices=4)
        tanh_x(1, slices=2)
        tanh_x(2)
        for g in range(3):
            cheb_pre[("A", g, 0)] = fill(xb_tiles[g], "A", g, 0)

        # ------------------------------------------------------------------
        # segment schedule: S0=A{0,1,2}, A3, B0, A4, B1, ..., A15, B12..B15
        emit_A([0, 1, 2], fill_ahead=1, hooks={
            0: [lambda: dma_x(3)],
            4: [lambda: tanh_x(3), lambda: dma_x(4)],
            5: [lambda: cheb_pre.__setitem__(("A", 3, 0),
                                             fill(xb_tiles[3], "A", 3, 0))],
            6: [lambda: cheb_pre.__setitem__(("A", 3, 1),
                                             fill(xb_tiles[3], "A", 3, 1))],
            8: [lambda: emit_LN(0)],
        })

        def A_hooks(g):
            h = {}
            add = lambda i, fn: h.setdefault(i, []).append(fn)
            if g == 3:
                add(1, lambda: emit_LN(1))
                add(4, lambda: emit_LN(2))
            else:
                add(1, lambda: emit_LN(g - 1))
            if g >= 4:
                add(2, lambda: emit_evac(g - 4))
            if g + 1 <= 15:
                add(4, lambda: tanh_x(g + 1))
            if g + 2 <= 15:
                add(4, lambda: dma_x(g + 2))
            # prefill for B_{g-3}, which directly follows this segment
            add(5, lambda: cheb_pre.__setitem__(
                ("B", g - 3, 0), fill(hb_tiles[g - 3], "B", g - 3, 0)))
            add(6, lambda: cheb_pre.__setitem__(
                ("B", g - 3, 1), fill(hb_tiles[g - 3], "B", g - 3, 1)))
            return h

        def B_hooks(k):
            h = {}
            add = lambda i, fn: h.setdefault(i, []).append(fn)
            if k + 4 <= 15:
                # prefill for A_{k+4}, which directly follows this segment
                add(5, lambda: cheb_pre.__setitem__(
                    ("A", k + 4, 0), fill(xb_tiles[k + 4], "A", k + 4, 0)))
                add(6, lambda: cheb_pre.__setitem__(
                    ("A", k + 4, 1), fill(xb_tiles[k + 4], "A", k + 4, 1)))
            if k == 12:
                add(3, lambda: cheb_pre.__setitem__(
                    ("B", 13, 0), fill(hb_tiles[13], "B", 13, 0)))
                add(4, lambda: cheb_pre.__setitem__(
                    ("B", 13, 1), fill(hb_tiles[13], "B", 13, 1)))
            if k == 13:
                # LN(15) here (not at B12): writing hb(15) recycles hb(12)'s
                # slot, whose readers (B12's fills) must all be emitted first.
                add(1, lambda: emit_LN(15))
            if k in (13, 14):
                add(5, lambda: cheb_pre.__setitem__(
                    ("B", k + 1, 0), fill(hb_tiles[k + 1], "B", k + 1, 0)))
                add(6, lambda: cheb_pre.__setitem__(
                    ("B", k + 1, 1), fill(hb_tiles[k + 1], "B", k + 1, 1)))
            if k >= 13:
                add(2, lambda: emit_evac(k - 1))
            if k == 15:
                add(8, lambda: emit_evac(15))
            return h

        emit_A([3], A_hooks(3))
        for k in range(12):
            emit_B(k, B_hooks(k))
            emit_A([k + 4], A_hooks(k + 4))
        for k in range(12, 16):
            emit_B(k, B_hooks(k))


_PROGRAMS = {}


def _get_program(trivial_affine: bool):
    key = trivial_affine
    if key in _PROGRAMS:
        return _PROGRAMS[key]
    nc = bacc.Bacc("TRN2", target_bir_lowering=False, debug=False,
                   num_devices=N_CORES)
    x_d = nc.dram_tensor("x_in", [BC, D0], F32, kind="ExternalInput").ap()
    w1_d = nc.dram_tensor("w1", [8, 128, 6, D1], BF16, kind="ExternalInput").ap()
    w2_d = nc.dram_tensor("w2", [8, 128, 6, D2], BF16, kind="ExternalInput").ap()
    b1_d = nc.dram_tensor("b1e", [D1], F32, kind="ExternalInput").ap()
    b2_d = nc.dram_tensor("b2e", [D2], F32, kind="ExternalInput").ap()
    if trivial_affine:
        g_d = be_d = None
    else:
        g_d = nc.dram_tensor("gam", [D1], BF16, kind="ExternalInput").ap()
        be_d = nc.dram_tensor("bet", [D1], BF16, kind="ExternalInput").ap()
    out_d = nc.dram_tensor("out", [BC, D2], F32, kind="ExternalOutput").ap()

    with tile.TileContext(nc) as tc:
        _kernel_body(tc, out_d, x_d, w1_d, w2_d, b1_d, b2_d, g_d, be_d)
    nc.compile()
    _PROGRAMS[key] = nc
    return nc


def _prep_inputs(x, coeff1, base_w1, bias1, ln_gamma, ln_beta, coeff2,
                 base_w2, bias2):
    x = np.ascontiguousarray(np.asarray(x, np.float32))
    coeff1 = np.asarray(coeff1, np.float32)
    coeff2 = np.asarray(coeff2, np.float32)

    # layout: [i_block, row_in_block, d, out] so each i-block is one
    # contiguous 1.5MB DMA transfer.
    w1 = np.empty((8, 128, 6, D1), ml_dtypes.bfloat16)
    w1[:, :, 0, :] = np.asarray(base_w1, np.float32).T.reshape(8, 128, D1)
    for d in range(1, 6):
        w1[:, :, d, :] = coeff1[:, :, d].T.reshape(8, 128, D1)
    w2 = np.empty((8, 128, 6, D2), ml_dtypes.bfloat16)
    w2[:, :, 0, :] = np.asarray(base_w2, np.float32).T.reshape(8, 128, D2)
    for d in range(1, 6):
        w2[:, :, d, :] = coeff2[:, :, d].T.reshape(8, 128, D2)
    b1e = (np.asarray(bias1, np.float32)
           + coeff1[:, :, 0].sum(axis=1)).astype(np.float32)
    b2e = (np.asarray(bias2, np.float32)
           + coeff2[:, :, 0].sum(axis=1)).astype(np.float32)

    g = np.asarray(ln_gamma, np.float32)
    be = np.asarray(ln_beta, np.float32)
    trivial = bool(np.all(g == 1.0) and np.all(be == 0.0))

    shared = {"w1": w1, "w2": w2, "b1e": b1e, "b2e": b2e}
    if not trivial:
        shared["gam"] = g.astype(ml_dtypes.bfloat16)
        shared["bet"] = be.astype(ml_dtypes.bfloat16)
    in_maps = []
    for cid in range(N_CORES):
        m = dict(shared)
        m["x_in"] = np.ascontiguousarray(x[cid * BC:(cid + 1) * BC])
        in_maps.append(m)
    return trivial, in_maps


def kernel_run(trace=False, **inputs):
    trivial, in_maps = _prep_inputs(**inputs)
    nc = _get_program(trivial)
    res = run_bass_kernel_spmd(nc, in_maps, core_ids=list(range(N_CORES)),
                               trace=trace)
    out = np.concatenate([r["out"] for r in res.results], axis=0)
    return out, res


def kernel(**inputs):
    out, _ = kernel_run(trace=False, **inputs)
    return out
